# revision 1
# baseline (speedup 1.0000x reference)
"""Self-contained Trainium2 Bass kernel for deformable conv 2d.

kernel(x, offset, weight) -> out, matching the jax reference:
  x[2,256,64,64] f32, offset[2,18,64,64] f32, weight[256,256,3,3] f32
  -> out[2,256,64,64] f32 (KH=KW=3, stride=1, pad=1, dil=1, DG=1).

Runs SPMD on 8 NeuronCores, data-parallel: core = (batch, spatial quarter).
See build_core_kernel docstring for the device-side pipeline.
"""

import sys

for _p in ("/opt/trn_rl_repo",):
    if _p not in sys.path:
        sys.path.insert(0, _p)



import numpy as np
import ml_dtypes

import concourse.bass as bass
import concourse.mybir as mybir
import concourse.tile as tile

F32 = mybir.dt.float32
BF16 = mybir.dt.bfloat16
I32 = mybir.dt.int32

N, CIN, H, W = 2, 256, 64, 64
COUT = 256
KH = KW = 3
K = KH * KW
S = H * W            # 4096 output positions per batch
SLOC = S // 4        # 1024 per core
TPC = 8              # ts slots per tap (SLOC/128)
NT = K * TPC         # 72 slots of [128 samples]

AluOp = mybir.AluOpType


def build_core_kernel(nc, tc, outs, ins):
    """Emit the per-core kernel. ins/outs are dicts of DRAM APs."""
    import os
    from contextlib import ExitStack

    xi = ins["xi"]          # [4096, 512] bf16 y-pair-interleaved image
    wT = ins["wT"]          # [2304, 256] bf16 lhsT
    maps_in = ins["maps_in"]  # [4, 128, 72] f32: offy, offx, basey, basex
    ident = ins["ident"]    # [128, 128] bf16 identity
    out = outs["out"]       # [128, 2, 1024] f32

    ctx = ExitStack()
    mp = ctx.enter_context(tc.tile_pool(name="maps", bufs=1))
    gp = ctx.enter_context(tc.tile_pool(name="gather", bufs=16))
    cp = ctx.enter_context(tc.tile_pool(name="colsrow", bufs=8))
    rp = ctx.enter_context(tc.tile_pool(name="rhsT", bufs=1))
    pp = ctx.enter_context(tc.tile_pool(name="psum", bufs=1, space="PSUM"))
    tp = ctx.enter_context(tc.tile_pool(name="tpsum", bufs=4, space="PSUM"))
    sp = ctx.enter_context(tc.tile_pool(name="static", bufs=1))

    v = nc.vector

    # ---- static loads (map inputs first: they gate the gather indices) ----
    mtile = sp.tile([128, 4, NT], F32, name="mtile")
    nc.sync.dma_start(mtile[:], maps_in.rearrange("a p t -> p a t"))
    oy, ox, by, bx = (mtile[:, i] for i in range(4))

    wT_s = sp.tile([128, 18, 256], BF16, name="wT_s")
    nc.sync.dma_start(wT_s[:], wT.rearrange("(j p) o -> p j o", p=128))
    id_s = sp.tile([128, 128], BF16, name="id_s")
    nc.sync.dma_start(id_s[:], ident)

    # ---- map computation in [128, 72] f32 (coords shifted by +128) ----
    def t72(name, dt=F32):
        return mp.tile([128, NT], dt, name=name)

    def floor_frac(base, off, pfx):
        """f = floor(p)+128 (exact int-valued f32), l = frac, h = 1-l."""
        t = t72(pfx + "_t")
        v.tensor_tensor(t[:], base, off, AluOp.add)
        v.tensor_scalar_add(t[:], t[:], 128.0)
        ti = t72(pfx + "_i", I32)
        v.tensor_copy(ti[:], t[:])
        tf = t72(pfx + "_f")
        v.tensor_copy(tf[:], ti[:])
        gt = t72(pfx + "_g")
        v.tensor_tensor(gt[:], tf[:], t[:], AluOp.is_gt)
        f = t72(pfx + "_fl")
        v.tensor_tensor(f[:], tf[:], gt[:], AluOp.subtract)
        return f, t

    def frac_of(f, t, pfx):
        l = t72(pfx + "_l")
        v.tensor_tensor(l[:], t[:], f[:], AluOp.subtract)
        h = t72(pfx + "_h")
        v.tensor_scalar(h[:], l[:], -1.0, 1.0, AluOp.mult, AluOp.add)
        return l, h

    fy, ty_ = floor_frac(by, oy, "y")
    fx, tx_ = floor_frac(bx, ox, "x")

    # window starts + gather rows first, so gathers launch early
    cys = t72("y_c")
    v.tensor_scalar(cys[:], fy[:], 128.0, 190.0, AluOp.max, AluOp.min)
    cxs = t72("x_c")
    v.tensor_scalar(cxs[:], fx[:], 128.0, 190.0, AluOp.max, AluOp.min)
    ridx = sp.tile([128, NT], I32, name="ridx")
    rf = t72("rf")
    v.scalar_tensor_tensor(rf[:], cys[:], 64.0, cxs[:], AluOp.mult, AluOp.add)
    v.tensor_scalar_add(rf[:], rf[:], -8320.0)
    v.tensor_copy(ridx[:], rf[:])

    # fractions only feed the slot weights -> computed after ridx
    ly, hy = frac_of(fy, ty_, "y")
    lx, hx = frac_of(fx, tx_, "x")

    def slot_weights(f, l, h, c, pfx):
        """Blend weights of window slots 0/1 with validity folded in."""
        # corner validity (valid range 128..191)
        cv = t72(pfx + "_cv")
        v.tensor_scalar(cv[:], f[:], 128.0, 191.0, AluOp.max, AluOp.min)
        v0 = t72(pfx + "_v0")
        v.tensor_tensor(v0[:], cv[:], f[:], AluOp.is_equal)
        f1 = t72(pfx + "_f1")
        v.tensor_scalar_add(f1[:], f[:], 1.0)
        cv1 = t72(pfx + "_cv1")
        v.tensor_scalar(cv1[:], f1[:], 128.0, 191.0, AluOp.max, AluOp.min)
        v1 = t72(pfx + "_v1")
        v.tensor_tensor(v1[:], cv1[:], f1[:], AluOp.is_equal)
        w0 = t72(pfx + "_w0")   # corner-0 weight (h * valid0)
        v.tensor_tensor(w0[:], h[:], v0[:], AluOp.mult)
        w1 = t72(pfx + "_w1")   # corner-1 weight (l * valid1)
        v.tensor_tensor(w1[:], l[:], v1[:], AluOp.mult)
        c1 = t72(pfx + "_c1")
        v.tensor_scalar_add(c1[:], c[:], 1.0)
        # slot selectors
        wt = []
        for sl, cs in ((0, c), (1, c1)):
            e0 = t72(f"{pfx}_e{sl}0")
            v.tensor_tensor(e0[:], cs[:], f[:], AluOp.is_equal)
            e1 = t72(f"{pfx}_e{sl}1")
            v.tensor_tensor(e1[:], cs[:], f1[:], AluOp.is_equal)
            v.tensor_tensor(e0[:], e0[:], w0[:], AluOp.mult)
            v.tensor_tensor(e1[:], e1[:], w1[:], AluOp.mult)
            wsl = t72(f"{pfx}_ws{sl}")
            v.tensor_tensor(wsl[:], e0[:], e1[:], AluOp.add)
            wt.append(wsl)
        return wt[0], wt[1]

    wt0, wt1 = slot_weights(fy, ly, hy, cys, "y")
    ws0, ws1 = slot_weights(fx, lx, hx, cxs, "x")

    # blend weights for slots [T0, B0, T1, B1]
    cw = []
    for name, a, b in (("cT0", wt0, ws0), ("cB0", wt1, ws0),
                       ("cT1", wt0, ws1), ("cB1", wt1, ws1)):
        t = sp.tile([128, NT], F32, name=name)
        v.tensor_tensor(t[:], a[:], b[:], AluOp.mult)
        cw.append(t)

    # ---- per-slot: gather + blend + transpose; per-tap matmuls ----
    rhsT = [rp.tile([128, 2, SLOC], BF16, name=f"rhsT{k}") for k in range(K)]
    ps = [pp.tile([128, SLOC], F32, name=f"psum{h}") for h in range(2)]
    osb = sp.tile([128, 2, SLOC], F32, name="osb")
    pts = []

    for t in range(NT):
        k, ts = t // TPC, t % TPC

        vt = gp.tile([128, 4, 256], BF16, name="vt")
        nc.gpsimd.indirect_dma_start(
            out=vt[:].rearrange("p a b -> p (a b)"),
            out_offset=None,
            in_=xi,
            in_offset=bass.IndirectOffsetOnAxis(ap=ridx[:, t : t + 1], axis=0),
        )

        # blend x-pairs on DVE (two independent 2-op chains + final add)
        cr = cp.tile([128, 256], BF16, name="colsrow")
        tm = cp.tile([128, 256], BF16, name="crtmp")
        v.tensor_scalar(cr[:], vt[:, 0, :], cw[0][:, t : t + 1], None, AluOp.mult)
        v.scalar_tensor_tensor(
            cr[:], vt[:, 2, :], cw[2][:, t : t + 1], cr[:], AluOp.mult, AluOp.add
        )
        v.tensor_scalar(tm[:], vt[:, 1, :], cw[1][:, t : t + 1], None, AluOp.mult)
        v.scalar_tensor_tensor(
            tm[:], vt[:, 3, :], cw[3][:, t : t + 1], tm[:], AluOp.mult, AluOp.add
        )
        v.tensor_tensor(cr[:], cr[:], tm[:], AluOp.add)

        tsl = ts % 4
        if tsl == 0:
            pt = tp.tile([128, 2, 4, 128], BF16, name="tpsum", space="PSUM")
            pts.append(pt)
        pt = pts[-1]
        for ch in range(2):
            nc.tensor.matmul(
                pt[:, ch, tsl, :], cr[:, ch * 128 : (ch + 1) * 128], id_s[:],
                is_transpose=True, start=True, stop=True,
            )
        if tsl == 3:
            ts0 = ts - 3
            nc.scalar.copy(
                rhsT[k][:, :, ts0 * 128 : (ts0 + 4) * 128].rearrange(
                    "p a (c b) -> p a c b", c=4
                ),
                pt[:],
            )

        if ts == TPC - 1:
            for h in range(2):
                for ch in range(2):
                    j = 2 * k + ch
                    for sh in range(2):
                        nc.tensor.matmul(
                            ps[h][:, sh * 512 : (sh + 1) * 512],
                            wT_s[:, j, h * 128 : (h + 1) * 128],
                            rhsT[k][:, ch, sh * 512 : (sh + 1) * 512],
                            start=(j == 0),
                            stop=(j == 17),
                        )
                if k == K - 1:
                    # stream each output half out as soon as its PSUM
                    # region finishes, overlapping the other half's matmuls
                    nc.scalar.copy(osb[:, h, :], ps[h][:])
                    nc.sync.dma_start(out[:, h, :], osb[:, h, :])

    ctx.close()


# ---------------- host-side prep ----------------

def core_inputs(x, offset, weight):
    """Full inputs (np f32) -> list of 8 per-core input dicts."""
    bf = ml_dtypes.bfloat16
    x = np.asarray(x, np.float32)
    offset = np.asarray(offset, np.float32)
    weight = np.asarray(weight, np.float32)

    # y-pair-interleaved channels-last images, bf16: xi[r] = [x[r], x[r+64]]
    xis = []
    for n in range(N):
        xcl = np.ascontiguousarray(x[n].reshape(CIN, S).T)  # [4096, 256]
        xi = np.zeros((S, 2 * CIN), np.float32)
        xi[:, :CIN] = xcl
        xi[: S - W, CIN:] = xcl[W:]
        xis.append(xi.astype(bf))

    # lhsT [k*256+c, o]
    wk = weight.reshape(COUT, CIN, K)           # [o, c, k]
    wT = np.ascontiguousarray(wk.transpose(2, 1, 0).reshape(K * CIN, COUT)).astype(bf)

    ident = np.eye(128, dtype=bf)

    off = offset.reshape(N, K, 2, S)
    ky, kx = np.meshgrid(np.arange(KH), np.arange(KW), indexing="ij")
    ky = ky.reshape(K); kx = kx.reshape(K)
    ho, wo = np.meshgrid(np.arange(H), np.arange(W), indexing="ij")
    base_y = (ho.reshape(S)[None, :] - 1 + ky[:, None]).astype(np.float32)  # [K,S]
    base_x = (wo.reshape(S)[None, :] - 1 + kx[:, None]).astype(np.float32)

    ins = []
    for core in range(8):
        n, qtr = core // 4, core % 4
        sl = slice(qtr * SLOC, (qtr + 1) * SLOC)

        def lay(a):  # [K, S] -> [128, 72]: [p, k*8+ts] = a[k, p*8+ts]
            aq = a[:, sl].reshape(K, 128, TPC)      # [k, p, ts]
            return np.ascontiguousarray(
                aq.transpose(1, 0, 2).reshape(128, NT)
            ).astype(np.float32)

        ins.append({
            "xi": xis[n],
            "wT": wT,
            "ident": ident,
            "maps_in": np.stack([
                lay(off[n, :, 0]), lay(off[n, :, 1]),
                lay(base_y), lay(base_x),
            ]),
        })
    return ins


def assemble(results):
    """list of 8 per-core {'out': [128,2,1024] f32} -> [2,256,64,64] f32."""
    out = np.zeros((N, COUT, S), np.float32)
    for core in range(8):
        n, qtr = core // 4, core % 4
        o = np.asarray(results[core]["out"])          # [128, 2, 1024]
        o = o.transpose(1, 0, 2).reshape(COUT, SLOC)  # [o, s'] s' = ts*128+p
        o = o.reshape(COUT, TPC, 128).transpose(0, 2, 1).reshape(COUT, SLOC)
        out[n, :, qtr * SLOC : (qtr + 1) * SLOC] = o
    return out.reshape(N, COUT, H, W)


def declare_io(nc):
    ins = {
        "xi": nc.dram_tensor("xi", [S, 2 * CIN], BF16, kind="ExternalInput").ap(),
        "wT": nc.dram_tensor("wT", [K * CIN, COUT], BF16, kind="ExternalInput").ap(),
        "ident": nc.dram_tensor("ident", [128, 128], BF16, kind="ExternalInput").ap(),
        "maps_in": nc.dram_tensor(
            "maps_in", [4, 128, NT], F32, kind="ExternalInput"
        ).ap(),
    }
    outs = {
        "out": nc.dram_tensor("out", [128, 2, SLOC], F32, kind="ExternalOutput").ap(),
    }
    return outs, ins


def build_module():
    from concourse import bacc

    nc = bacc.Bacc("TRN2", target_bir_lowering=False, debug=False, num_devices=8)
    outs, ins = declare_io(nc)
    with tile.TileContext(nc) as tc:
        build_core_kernel(nc, tc, outs, ins)
    nc.compile()
    return nc


_NC_CACHE = []


def kernel(x, offset, weight):
    """Full (unsharded) inputs -> full output, computed on 8 NeuronCores."""
    import time

    from concourse.bass_utils import run_bass_kernel_spmd

    if not _NC_CACHE:
        _NC_CACHE.append(build_module())
    nc = _NC_CACHE[0]
    core_ins = core_inputs(x, offset, weight)
    last = None
    for attempt in range(3):
        try:
            res = run_bass_kernel_spmd(nc, core_ins, core_ids=list(range(8)))
            return assemble(res.results)
        except Exception as e:  # transient device-session failures
            last = e
            time.sleep(2.0 * (attempt + 1))
    raise last



# revision 2
# speedup vs baseline: 1.1063x; 1.1063x over previous
"""Self-contained Trainium2 Bass kernel for deformable conv 2d.

kernel(x, offset, weight) -> out, matching the jax reference:
  x[2,256,64,64] f32, offset[2,18,64,64] f32, weight[256,256,3,3] f32
  -> out[2,256,64,64] f32 (KH=KW=3, stride=1, pad=1, dil=1, DG=1).

Runs SPMD on 8 NeuronCores, data-parallel: core = (batch, spatial quarter).

Device pipeline (per core, per ts-group g of 128 output positions):
  - 9 indirect gathers (one per tap) fetch each sample's 2x2 bilinear
    window (4 corners x 256ch bf16) into [128 pos, 4*256].
  - DVE builds diag(w_corner) tiles from host-computed bilinear weights
    (one broadcast-AP tensor_tensor per group).
  - PE does blend+transpose in one step: psum[ch,pos] += vt_a^T @ diag(w_a)
    accumulated over the 4 corners (a regular matmul with diagonal rhs
    scales each transposed column by its sample weight).
  - Act copies blended psum -> SBUF bf16 rhsT; PE contracts the 18
    (tap, cin-chunk) pieces with the conv weights into psum out.
Host precomputes gather indices and corner weights from the offsets.
"""

import sys

for _p in ("/opt/trn_rl_repo",):
    if _p not in sys.path:
        sys.path.insert(0, _p)


import numpy as np
import ml_dtypes

import concourse.bass as bass
import concourse.mybir as mybir
import concourse.tile as tile

F32 = mybir.dt.float32
BF16 = mybir.dt.bfloat16
I32 = mybir.dt.int32

N, CIN, H, W = 2, 256, 64, 64
COUT = 256
KH = KW = 3
K = KH * KW
S = H * W            # 4096 output positions per batch
SLOC = S // 4        # 1024 per core
NG = 8               # ts-groups per core (128 positions each)
NT = K * NG          # 72 (tap, group) slots

AluOp = mybir.AluOpType


def build_core_kernel(nc, tc, outs, ins):
    """Emit the per-core kernel. ins/outs are dicts of DRAM APs."""
    from contextlib import ExitStack

    xi = ins["xi"]          # [4096, 512] bf16 y-pair-interleaved image
    wT = ins["wT"]          # [2304, 256] bf16 lhsT
    ridx_d = ins["ridx"]    # [128, 72] i32 gather rows, col = g*9+k
    cw_d = ins["cw"]        # [128, 8, 36] bf16 corner weights (g, k*4+a)
    ident_d = ins["ident"]  # [128, 128] bf16 identity
    out = outs["out"]       # [128, 8, 2, 128] f32

    ctx = ExitStack()
    sp = ctx.enter_context(tc.tile_pool(name="static", bufs=1))
    gp = ctx.enter_context(tc.tile_pool(name="gather", bufs=14))
    dgp = ctx.enter_context(tc.tile_pool(name="diag", bufs=2))
    rp = ctx.enter_context(tc.tile_pool(name="rhsT", bufs=2))
    bp = ctx.enter_context(tc.tile_pool(name="bpsum", bufs=2, space="PSUM"))
    cp = ctx.enter_context(tc.tile_pool(name="cpsum", bufs=2, space="PSUM"))
    op = ctx.enter_context(tc.tile_pool(name="osb", bufs=2))

    # ---- static loads ----
    ridx = sp.tile([128, NT], I32, name="ridx")
    nc.sync.dma_start(ridx[:], ridx_d)
    cw = sp.tile([128, NG, 4 * K], BF16, name="cw")
    nc.sync.dma_start(cw[:], cw_d)
    ident = sp.tile([128, 128], BF16, name="ident")
    nc.sync.dma_start(ident[:], ident_d)
    wT_s = sp.tile([128, 18, 256], BF16, name="wT_s")
    nc.sync.dma_start(wT_s[:], wT.rearrange("(j p) o -> p j o", p=128))

    id_bc = ident[:].unsqueeze(1).broadcast_to([128, 4 * K, 128])

    for g in range(NG):
        # diag tiles for the whole group: diag[p, (k,a), f] = I[p,f]*cw[p,g,(k,a)]
        diag = dgp.tile([128, 4 * K, 128], BF16, name="diag")
        cw_bc = cw[:, g].unsqueeze(-1).broadcast_to([128, 4 * K, 128])
        nc.vector.tensor_tensor(diag[:], id_bc, cw_bc, AluOp.mult)

        rhsT = rp.tile([128, K, 2, 128], BF16, name="rhsT")
        po = cp.tile([128, 2, 128], F32, name="po", space="PSUM")

        for k3 in range(3):            # taps in batches of 3
            pb = bp.tile([128, 3, 2, 128], F32, name="pb", space="PSUM")
            for kk in range(3):
                k = k3 * 3 + kk
                gt = gp.tile([128, 4, 256], BF16, name="gt")
                t = g * K + k
                nc.gpsimd.indirect_dma_start(
                    out=gt[:].rearrange("p a b -> p (a b)"),
                    out_offset=None,
                    in_=xi,
                    in_offset=bass.IndirectOffsetOnAxis(
                        ap=ridx[:, t : t + 1], axis=0
                    ),
                )
                # blend + transpose: psum[ch,pos] += gt_a^T @ diag(w_a)
                for cc in range(2):
                    for a in range(4):
                        nc.tensor.matmul(
                            pb[:, kk, cc, :],
                            gt[:, a, cc * 128 : (cc + 1) * 128],
                            diag[:, 4 * k + a, :],
                            start=(a == 0),
                            stop=(a == 3),
                        )
            nc.scalar.copy(rhsT[:, 3 * k3 : 3 * k3 + 3], pb[:])
            # conv: accumulate the 6 (tap, cin-chunk) pieces into psum out
            for kk in range(3):
                k = k3 * 3 + kk
                for h in range(2):
                    for cc in range(2):
                        j = 2 * k + cc
                        nc.tensor.matmul(
                            po[:, h, :],
                            wT_s[:, j, h * 128 : (h + 1) * 128],
                            rhsT[:, k, cc, :],
                            start=(j == 0 and h == 0),
                            stop=(j == 17 and h == 1),
                        )
        osb = op.tile([128, 2, 128], F32, name="osb")
        nc.scalar.copy(osb[:], po[:])
        nc.sync.dma_start(out[:, g], osb[:])

    ctx.close()


# ---------------- host-side prep ----------------

def core_inputs(x, offset, weight):
    """Full inputs (np f32) -> list of 8 per-core input dicts."""
    bf = ml_dtypes.bfloat16
    x = np.asarray(x, np.float32)
    offset = np.asarray(offset, np.float32)
    weight = np.asarray(weight, np.float32)

    # y-pair-interleaved channels-last images, bf16: xi[r] = [x[r], x[r+64]]
    xis = []
    for n in range(N):
        xcl = np.ascontiguousarray(x[n].reshape(CIN, S).T)  # [4096, 256]
        xi = np.zeros((S, 2 * CIN), np.float32)
        xi[:, :CIN] = xcl
        xi[: S - W, CIN:] = xcl[W:]
        xis.append(xi.astype(bf))

    # lhsT [k*256+c, o]
    wk = weight.reshape(COUT, CIN, K)           # [o, c, k]
    wT = np.ascontiguousarray(wk.transpose(2, 1, 0).reshape(K * CIN, COUT)).astype(bf)

    ident = np.eye(128, dtype=bf)

    # sample coordinates: py/px [K, S] per batch
    off = offset.reshape(N, K, 2, S)
    ky, kx = np.meshgrid(np.arange(KH), np.arange(KW), indexing="ij")
    ky = ky.reshape(K, 1).astype(np.float32)
    kx = kx.reshape(K, 1).astype(np.float32)
    ho, wo = np.meshgrid(np.arange(H), np.arange(W), indexing="ij")
    base_y = ho.reshape(1, S).astype(np.float32) - 1.0 + ky   # [K, S]
    base_x = wo.reshape(1, S).astype(np.float32) - 1.0 + kx

    ins = []
    for core in range(8):
        n, qtr = core // 4, core % 4
        sl = slice(qtr * SLOC, (qtr + 1) * SLOC)
        py = base_y[:, sl] + off[n, :, 0, sl]   # [K, 1024]
        px = base_x[:, sl] + off[n, :, 1, sl]

        fy = np.floor(py)
        fx = np.floor(px)
        ly, lx = py - fy, px - fx
        hy, hx = 1.0 - ly, 1.0 - lx
        wy_c = np.clip(fy, 0.0, 62.0)           # window start rows
        wx_c = np.clip(fx, 0.0, 62.0)

        def sw(f, l, h, wc):
            """weights of window slots 0/1 along one axis, validity folded."""
            v0 = (f >= 0) & (f <= 63)
            v1 = (f + 1 >= 0) & (f + 1 <= 63)
            w0 = h * v0                          # corner f
            w1 = l * v1                          # corner f+1
            ws = []
            for s_ in (0, 1):
                c = wc + s_
                ws.append(w0 * (c == f) + w1 * (c == f + 1))
            return ws                            # [2][K, 1024]

        wys = sw(fy, ly, hy, wy_c)
        wxs = sw(fx, lx, hx, wx_c)
        rows = (wy_c * 64.0 + wx_c).astype(np.int32)        # [K, 1024]

        # device layouts: position p*8+g <-> (partition p, group g)
        def lay(a):                               # [K, 1024] -> [128, 8, K]
            return np.ascontiguousarray(
                a.reshape(K, 128, NG).transpose(1, 2, 0)
            )

        ridx = lay(rows).reshape(128, NG * K).astype(np.int32)  # col g*9+k
        cwa = np.stack(
            [wys[0] * wxs[0], wys[1] * wxs[0], wys[0] * wxs[1], wys[1] * wxs[1]],
            axis=0,
        )                                          # [4, K, 1024]
        # -> [128, 8, K*4+a]
        cw = np.ascontiguousarray(
            cwa.reshape(4, K, 128, NG).transpose(2, 3, 1, 0).reshape(128, NG, 4 * K)
        ).astype(bf)

        ins.append({
            "xi": xis[n],
            "wT": wT,
            "ident": ident,
            "ridx": ridx,
            "cw": cw,
        })
    return ins


def assemble(results):
    """list of 8 per-core {'out': [128,8,2,128] f32} -> [2,256,64,64] f32."""
    out = np.zeros((N, COUT, S), np.float32)
    for core in range(8):
        n, qtr = core // 4, core % 4
        o = np.asarray(results[core]["out"])          # [oc, g, h, p]
        o = o.transpose(2, 0, 3, 1)                   # [h, oc, p, g]
        o = o.reshape(COUT, SLOC)                     # [cout, p*8+g]
        out[n, :, qtr * SLOC : (qtr + 1) * SLOC] = o
    return out.reshape(N, COUT, H, W)


def declare_io(nc):
    ins = {
        "xi": nc.dram_tensor("xi", [S, 2 * CIN], BF16, kind="ExternalInput").ap(),
        "wT": nc.dram_tensor("wT", [K * CIN, COUT], BF16, kind="ExternalInput").ap(),
        "ridx": nc.dram_tensor("ridx", [128, NT], I32, kind="ExternalInput").ap(),
        "cw": nc.dram_tensor("cw", [128, NG, 4 * K], BF16, kind="ExternalInput").ap(),
        "ident": nc.dram_tensor("ident", [128, 128], BF16, kind="ExternalInput").ap(),
    }
    outs = {
        "out": nc.dram_tensor(
            "out", [128, NG, 2, 128], F32, kind="ExternalOutput"
        ).ap(),
    }
    return outs, ins


def build_module():
    from concourse import bacc

    nc = bacc.Bacc("TRN2", target_bir_lowering=False, debug=False, num_devices=8)
    outs, ins = declare_io(nc)
    with tile.TileContext(nc) as tc:
        build_core_kernel(nc, tc, outs, ins)
    nc.compile()
    return nc


_NC_CACHE = []


def kernel(x, offset, weight):
    """Full (unsharded) inputs -> full output, computed on 8 NeuronCores."""
    import time

    from concourse.bass_utils import run_bass_kernel_spmd

    if not _NC_CACHE:
        _NC_CACHE.append(build_module())
    nc = _NC_CACHE[0]
    core_ins = core_inputs(x, offset, weight)
    last = None
    for attempt in range(3):
        try:
            res = run_bass_kernel_spmd(nc, core_ins, core_ids=list(range(8)))
            return assemble(res.results)
        except Exception as e:  # transient device-session failures
            last = e
            time.sleep(2.0 * (attempt + 1))
    raise last
